# revision 23
# baseline (speedup 1.0000x reference)
"""BertSelfAttention Trainium2 kernel.

Full inputs in, full output out. Sharding: 8 cores = (batch b in {0,1}) x
(head-group hg in {0..3}); each core computes 4 heads of one batch and
produces the output feature slice out[b, :, hg*256:(hg+1)*256].

Per-core device program (all cores run the same NEFF, SPMD):
  xT [1024, 2048]      hidden_states[b].T  (host-packed p-major)
  QT/KT computed transposed [d, s] fp16 with bias, per-nb tiles
  V computed [s, d] fp16, rows scaled by exp(mask), plus a per-head
    ones*exp(mask) column so the ctx matmul also yields softmax row sums
  scoresT [k, q] tiles via fp16 matmuls; the two heads of a pair run as
    row-half MMs (tile_position 0/64) interleaved per-ktile so they
    execute CONCURRENTLY on the two PE row-halves (~2x scores rate)
  exp on ACT directly from PSUM (scale=1/8, bias=-4 folded in).  The
    kernel is ACT-bound: ~131us of exp is the floor, so everything else
    is scheduled to keep the ACT exp pipeline 100% fed.
  ctx[q, d] = expT.T @ [V|em] accumulated over 16 k-tiles, emitted as
    (head, qtile) chunks INTERLEAVED between scores batches so the PE
    has dense work while exp drains PSUM (HAM stays at full clock).
  Output written via one merged DMA per (hp, qb) unit on the gpsimd
    SWDGE queue; input DMAs are 128-256KB chunks with 2-4KB lines spread
    across the sync/scalar/gpsimd trigger queues.
"""

import numpy as np

B = 2
S = 2048
H = 1024
NH = 16
HD = 64

NCORES = 8
HPC = 4          # heads per core
DS = HPC * HD    # 256 output dims per core
FT = H // 128    # 8 f-tiles (contraction tiles for projections)
KT = S // 128    # 16 key tiles
ST = S // 128    # 16 s-tiles of V
QB = 4           # q blocks of 512
QBS = 512
VW = HPC * (HD + 1)  # 260: V columns + one em column per head

EXP_BIAS = -4.0  # uniform shift inside exp; cancels in softmax, guards fp16

_CACHE = {}


def _build_program(split_waits=True):
    import concourse.bass as bass
    import concourse.mybir as mybir
    import concourse.tile as tile
    from concourse.vector_clock import ScopedClock

    f32 = mybir.dt.float32
    f16 = mybir.dt.float16
    AF = mybir.ActivationFunctionType
    OP = mybir.AluOpType

    class SplitDrainTileContext(tile.TileContext):
        """The walrus build here rejects instructions with more than one
        sync wait ("Too many sync wait commands"); hoist excess waits onto
        preceding same-engine NOPs."""

        MAX_WAITS_PER_DRAIN = 1
        split_waits_enabled = True

        def _drain_and_barrier(self, tick_clock, wait_clock):
            drain_inst = self.nc.sync.drain()
            wait_clock.add_sem_waits(
                drain_inst.ins, ScopedClock({None: tick_clock.global_clock})
            )
            self.nc.all_engine_barrier()
            assert self.sems is not None
            popped = self.nc._tile_sem_poison_stack.pop()
            assert popped is self._sem_poison
            self.nc.clear_and_free_semaphores(list(self.sems.allocated().values()))
            self.nc.all_engine_barrier()
            if self.split_waits_enabled:
                self._split_multi_waits()

        def _split_multi_waits(self):
            k = self.MAX_WAITS_PER_DRAIN
            nc = self.nc
            for bb in nc.bb_map.values():
                il = bb.bb.instructions
                new = []
                for inst in il:
                    si = getattr(inst, "sync_info", None)
                    waits = list(si.on_wait) if si is not None and si.on_wait else []
                    if len(waits) > k:
                        for j in range(0, len(waits) - k, k):
                            nop = mybir.InstNoOp(
                                name=nc.get_next_instruction_name(),
                                engine=inst.engine,
                                sync_info=mybir.SyncInfo(
                                    on_wait=waits[j : j + k], on_update=[]
                                ),
                                bass_nofuse=True,
                            )
                            new.append(nop)
                        inst.sync_info = mybir.SyncInfo(
                            on_wait=waits[len(waits) - k :],
                            on_update=list(si.on_update) if si.on_update else [],
                        )
                    new.append(inst)
                il[:] = new

    nc = bass.Bass("TRN2", target_bir_lowering=False, debug=False,
                   num_devices=NCORES)

    # Host-packed p-major layouts (see _make_in_maps)
    xT_d = nc.dram_tensor("xT", [128, QB * FT * QBS], f16, kind="ExternalInput")
    wqT_d = nc.dram_tensor("wqT", [128, FT * DS], f16, kind="ExternalInput")
    wkT_d = nc.dram_tensor("wkT", [128, FT * DS], f16, kind="ExternalInput")
    wvT_d = nc.dram_tensor("wvT", [128, FT * VW], f16, kind="ExternalInput")
    bq_d = nc.dram_tensor("bq", [128, 2], f32, kind="ExternalInput")
    bk_d = nc.dram_tensor("bk", [128, 2], f32, kind="ExternalInput")
    bvb_d = nc.dram_tensor("bvb", [128, DS], f32, kind="ExternalInput")
    em_d = nc.dram_tensor("em", [128, KT], f32, kind="ExternalInput")
    # out: [p, qt(16), j(256)]; host unpermutes back to [S, DS]
    out_d = nc.dram_tensor("out", [128, 16, DS], f32, kind="ExternalOutput")

    SplitDrainTileContext.split_waits_enabled = split_waits
    with SplitDrainTileContext(nc) as tc:
        from contextlib import ExitStack

        with ExitStack() as ctx:
            const = ctx.enter_context(tc.tile_pool(name="const", bufs=1))
            qk = ctx.enter_context(tc.tile_pool(name="qk", bufs=1))
            vp = ctx.enter_context(tc.tile_pool(name="vp", bufs=1))
            epool = ctx.enter_context(tc.tile_pool(name="epool", bufs=1))
            opool = ctx.enter_context(tc.tile_pool(name="opool", bufs=1))
            rpool = ctx.enter_context(tc.tile_pool(name="rpool", bufs=1))

            # ---- constants ----
            bq_sb = const.tile([128, 2], f32, tag="bq", bufs=1, name="bq_sb")
            bk_sb = const.tile([128, 2], f32, tag="bk", bufs=1, name="bk_sb")
            bvb_sb = const.tile([128, DS], f32, tag="bvb", bufs=1, name="bvb_sb")
            em_sb = const.tile([128, KT], f32, tag="em", bufs=1, name="em_sb")
            ebias = const.tile([128, 1], f32, tag="ebias", bufs=1, name="ebias")
            warm = const.tile([128, 1], f32, tag="warm", bufs=1, name="warm")

            # ---- persistent activations: per-nb tiles so consumers start
            # as soon as their block is projected ----
            qt = [[qk.tile([128, QBS], f16, tag=f"qt{m}_{nb}", bufs=1,
                           name=f"qt{m}_{nb}") for nb in range(QB)]
                  for m in range(2)]
            kt_sb = [[qk.tile([128, QBS], f16, tag=f"kt{m}_{nb}", bufs=1,
                              name=f"kt{m}_{nb}") for nb in range(QB)]
                     for m in range(2)]
            vones = [vp.tile([128, VW], f16, tag=f"v{st}", bufs=1,
                             name=f"vones{st}") for st in range(ST)]

            # ---- input tiles: per-(ft, nb) x tiles so projection MMs
            # pipeline behind the DMA stream at tile granularity ----
            xw = ctx.enter_context(tc.tile_pool(name="xw", bufs=1))
            NBW = FT * QBS
            xt = [[xw.tile([128, QBS], f16, tag=f"xt{ft}_{nb}", bufs=1,
                           name=f"xt{ft}_{nb}") for nb in range(QB)]
                  for ft in range(FT)]
            wq_sb = xw.tile([128, FT * DS], f16, tag="wq", bufs=1, name="wq")
            wk_sb = xw.tile([128, FT * DS], f16, tag="wk", bufs=1, name="wk")
            wv_sb = xw.tile([128, FT * VW], f16, tag="wv", bufs=1, name="wv")

            def xt_v(ft, nb):
                return xt[ft][nb][:]

            def wq_v(ft):
                return wq_sb[:, ft * DS:(ft + 1) * DS]

            def wk_v(ft):
                return wk_sb[:, ft * DS:(ft + 1) * DS]

            def wv_v(ft):
                return wv_sb[:, ft * VW:(ft + 1) * VW]

            # ---- input DMAs: <=128KB transfers (per-queue BW is only
            # ~19GB/s, so many parallel queues beat few big transfers),
            # spread across the three DGE trigger queues in need-order ----
            def chunks(eng, dst, src, n):
                w = dst.shape[1]
                cw = w // n
                for i in range(n):
                    cs = slice(i * cw, (i + 1) * cw)
                    eng.dma_start(dst[:, cs], src[:, cs])

            def xt_dma(eng, ft, nb):
                cs = slice(nb * NBW + ft * QBS, nb * NBW + (ft + 1) * QBS)
                eng.dma_start(xt[ft][nb][:], xT_d.ap()[:, cs])

            nc.vector.memset(ebias[:], EXP_BIAS)
            # table load + warm first on scalar so it doesn't delay exp later
            nc.scalar.activation(warm[:], ebias[:], AF.Exp)

            chunks(nc.sync, wk_sb[:], wkT_d.ap(), 4)
            nc.sync.dma_start(bk_sb[:], bk_d.ap())
            nc.sync.dma_start(bq_sb[:], bq_d.ap())
            for ft in range(FT):
                xt_dma(nc.sync, ft, 0)
            for ft in range(4):
                xt_dma(nc.sync, ft, 3)
            chunks(nc.scalar, wq_sb[:], wqT_d.ap(), 4)
            for ft in range(FT):
                xt_dma(nc.scalar, ft, 1)
            nc.scalar.dma_start(em_sb[:], em_d.ap())
            chunks(nc.gpsimd, wv_sb[:], wvT_d.ap(), 4)
            for ft in range(FT):
                xt_dma(nc.gpsimd, ft, 2)
            for ft in range(4, FT):
                xt_dma(nc.gpsimd, ft, 3)
            nc.gpsimd.dma_start(bvb_sb[:], bvb_d.ap())

            # ---- PSUM pools: proj 1 + scores 2x3 + ctx 1 = 8 banks ----
            ps_pj = ctx.enter_context(
                tc.tile_pool(name="ps_pj", bufs=1, space="PSUM"))
            ps_sc = ctx.enter_context(
                tc.tile_pool(name="ps_sc", bufs=2, space="PSUM"))
            ps_cx = ctx.enter_context(
                tc.tile_pool(name="ps_cx", bufs=1, space="PSUM"))

            mm = nc.tensor.matmul

            # ---- work units ----
            def qk_proj_block(w_v, bias_sb, m, dst_tiles, nb):
                ps = ps_pj.tile([128, QBS], f32, tag="pj", name="pspj")
                for ft in range(FT):
                    mm(ps[:],
                       w_v(ft)[:, m * 128:(m + 1) * 128],
                       xt_v(ft, nb),
                       start=(ft == 0), stop=(ft == FT - 1))
                nc.vector.tensor_scalar_add(
                    dst_tiles[nb][:], ps[:], bias_sb[:, m:m + 1])

            def v_proj_block(st):
                nb, within = divmod(st, 4)
                ws = slice(within * 128, (within + 1) * 128)
                ps = ps_pj.tile([128, QBS], f32, tag="pj", name="pspjv")
                for ft in range(FT):
                    mm(ps[:, 0:VW],
                       xt_v(ft, nb)[:, ws],
                       wv_v(ft),
                       start=(ft == 0), stop=(ft == FT - 1))
                nc.vector.tensor_scalar_mul(
                    vones[st][:], ps[:, 0:VW], em_sb[:, st:st + 1])
                for hh in range(HPC):
                    c = hh * (HD + 1) + HD
                    nc.vector.tensor_copy(
                        vones[st][:, c:c + 1], em_sb[:, st:st + 1])

            BATCHES = [(0, 3), (3, 3), (6, 3), (9, 3), (12, 3), (15, 1)]

            def scores_batch(hp, qb, eA, eB, k0, nk, half=None, paired=False):
                """When paired, the A/B row-half MMs are interleaved
                per-ktile: tile_position (0,0)/(64,0) target disjoint PE
                row groups so the pair runs concurrently (~2x rate, but
                lower PE duty -> HAM throttle risk).  When not paired, the
                sides are emitted grouped (denser PE occupancy)."""
                w = nk * QBS
                es = slice(k0 * QBS, k0 * QBS + w)
                sides = [(0, eA, 0), (1, eB, 64)] if half is None else \
                    [[(0, eA, 0), (1, eB, 64)][half]]
                ps = {}
                for (side, e, p0) in sides:
                    ps[side] = ps_sc.tile([128, 3 * QBS], f32, tag="sc",
                                          name="pscA" if side == 0 else "pscB")

                def emit_mm(side, p0, j):
                    ktile = k0 + j
                    knb, kw = divmod(ktile, 4)
                    ks = slice(kw * 128, (kw + 1) * 128)
                    js = slice(j * QBS, (j + 1) * QBS)
                    mm(ps[side][:, js],
                       kt_sb[hp][knb][p0:p0 + 64, ks],
                       qt[hp][qb][p0:p0 + 64, :],
                       tile_position=(p0, 0))

                if paired:
                    for j in range(nk):
                        for (side, e, p0) in sides:
                            emit_mm(side, p0, j)
                else:
                    for (side, e, p0) in sides:
                        for j in range(nk):
                            emit_mm(side, p0, j)
                for (side, e, p0) in sides:
                    nc.scalar.activation(
                        e[:, es], ps[side][:, 0:w],
                        AF.Exp, bias=ebias[:], scale=0.125)

            def ctx_chunks(hp, qb, eA, eB):
                """Return (mm_thunks, evict_thunks).  mm thunk (a, qq) =
                16 accumulating PE MMs into the a-unit's PSUM tile; evict
                thunk (a, qq) = DVE reciprocal + normalize into the shared
                ot tile.  MMs are PE-only (no ACT contention) so they can
                interleave between scores batches; the DVE evictions (PSUM
                readers, which contend with the ACT exp) are emitted
                bunched at the iteration boundary.  a=0 evictions must
                precede a=1 MMs (single cx PSUM bank rotates)."""
                state = {}

                def chunk_mm(a, qq):
                    hh = 2 * hp + a
                    e = eA if a == 0 else eB
                    if a == 0 and qq == 0:
                        state["ot"] = opool.tile([128, 4 * 128], f32,
                                                 tag="ot", bufs=2, name="ot")
                    if qq == 0:
                        state[a] = ps_cx.tile([128, 4 * (HD + 1)], f32,
                                              tag="cx", name="cps")
                    cps = state[a][:, qq * (HD + 1):(qq + 1) * (HD + 1)]
                    for ktile in range(KT):
                        lo = ktile * QBS + qq * 128
                        mm(cps,
                           e[:, lo:lo + 128],
                           vones[ktile][:, hh * (HD + 1):(hh + 1) * (HD + 1)],
                           start=(ktile == 0), stop=(ktile == KT - 1))

                def chunk_evict(a, qq):
                    hh = 2 * hp + a
                    cps = state[a][:, qq * (HD + 1):(qq + 1) * (HD + 1)]
                    ot = state["ot"]
                    r = rpool.tile([128, 1], f32, tag="r", bufs=4, name="r")
                    nc.vector.reciprocal(r[:], cps[:, HD:HD + 1])
                    nc.vector.scalar_tensor_tensor(
                        ot[:, qq * 128 + a * 64:qq * 128 + (a + 1) * 64],
                        cps[:, 0:HD], r[:],
                        bvb_sb[:, hh * HD:(hh + 1) * HD],
                        op0=OP.mult, op1=OP.add)
                    if a == 1 and qq == 3:
                        nc.gpsimd.dma_start(
                            out_d.ap()[:, qb * 4:(qb + 1) * 4,
                                       hp * 128:(hp + 1) * 128],
                            ot[:])

                mms = [lambda a=a, qq=qq: chunk_mm(a, qq)
                       for a in (0, 1) for qq in range(4)]
                evs = [lambda a=a, qq=qq: chunk_evict(a, qq)
                       for a in (0, 1) for qq in range(4)]
                return mms, evs

            def k0_block(nb):
                qk_proj_block(wk_v, bk_sb, 0, kt_sb[0], nb)

            def q0_block(nb):
                qk_proj_block(wq_v, bq_sb, 0, qt[0], nb)

            def k1_block(nb):
                qk_proj_block(wk_v, bk_sb, 1, kt_sb[1], nb)

            def q1_block(nb):
                qk_proj_block(wq_v, bq_sb, 1, qt[1], nb)

            # ---- emission schedule ----
            # Per iteration: 6 scores batches interleaved with the previous
            # iteration's 8 ctx chunks + this iteration's filler blocks.
            # pre[j] = fillers that must precede batch j (projection deps);
            # post = fillers placed round-robin in the batch gaps.
            def emit_iteration(hp, qb, eA, eB, cmms, cevs, pre, slot_fill,
                               post, paired):
                # ctx MM chunks (PE-only, no ACT contention) interleave in
                # the gaps between scores batches; the DVE evictions (PSUM
                # readers that contend with the ACT exp) bunch into two
                # slots covered by the ACT batch buffer.  a=0 evictions
                # must precede a=1 MMs (the single cx PSUM bank rotates).
                slots = [[] for _ in range(len(BATCHES))]
                if cmms:
                    slots[0] = cmms[0:2]
                    slots[1] = cmms[2:4]
                    slots[2] = cevs[0:4]
                    slots[3] = cmms[4:6]
                    slots[4] = cmms[6:8]
                    slots[5] = cevs[4:8]
                if slot_fill:
                    per = -(-len(slot_fill) // len(BATCHES))
                    for j in range(len(BATCHES)):
                        slots[j] = list(slot_fill[j * per:(j + 1) * per]) \
                            + slots[j]
                for j, (k0, nk) in enumerate(BATCHES):
                    for u in pre.get(j, []):
                        u()
                    scores_batch(hp, qb, eA, eB, k0, nk, paired=paired)
                    for u in slots[j]:
                        u()
                for u in post:
                    u()

            fill_pre = {
                0: {1: [lambda: k0_block(1)],
                    2: [lambda: k0_block(2)],
                    4: [lambda: k0_block(3)]},
            }
            # it0 carries ALL V-proj blocks in its batch gaps (ordered by
            # xnb DMA arrival) so every vones tile exists before the first
            # ctx chunk of it1; proj fillers for later iterations go at
            # the boundary (their DVE evictions contend with exp the least
            # there, covered by the ACT batch buffer).
            fill_slot = {
                0: [lambda st=st: v_proj_block(st) for st in range(ST)]
                   + [lambda: q0_block(1)],
            }
            fill_post = {
                1: [lambda: q0_block(2)],
                2: [lambda: q0_block(3), lambda: k1_block(0),
                    lambda: k1_block(1), lambda: k1_block(2)],
                3: [lambda: k1_block(3), lambda: q1_block(0)],
                4: [lambda: q1_block(1)],
                5: [lambda: q1_block(2)],
                6: [lambda: q1_block(3)],
            }

            # prologue: first scores batch only needs K0-nb0 + Q0-nb0
            k0_block(0)
            q0_block(0)

            # all iterations run paired scores: the late iterations are
            # under the firmware clock clamp anyway (PE at 1.2GHz from
            # ~143us on), where efficiency beats density
            PAIRED_ITS = {0, 1, 2, 3, 4, 5, 6}

            prev = None
            for it in range(8):
                hp, qb = divmod(it, QB)
                eA = epool.tile([128, KT * QBS], f16, tag="eA", bufs=3,
                                name="eA")
                eB = epool.tile([128, KT * QBS], f16, tag="eB", bufs=3,
                                name="eB")
                cmms, cevs = ctx_chunks(*prev) if prev is not None else \
                    ([], [])
                if it < 7:
                    emit_iteration(hp, qb, eA, eB, cmms, cevs,
                                   fill_pre.get(it, {}),
                                   fill_slot.get(it, []),
                                   fill_post.get(it, []),
                                   it in PAIRED_ITS)
                else:
                    # last iteration: A-half batches first so eA completes
                    # early; ctx(it6) interleaves with the A batches, then
                    # ctx(it7,a=0) interleaves with the B batches, and only
                    # the a=1 chunks trail the final exp.
                    mm7, ev7 = ctx_chunks(hp, qb, eA, eB)
                    slots = [cmms[0:2], cmms[2:4], cevs[0:4],
                             cmms[4:6], cmms[6:8], cevs[4:8]]
                    for j, (k0, nk) in enumerate(BATCHES):
                        scores_batch(hp, qb, eA, eB, k0, nk, half=0)
                        for u in slots[j]:
                            u()
                    for j, (k0, nk) in enumerate(BATCHES):
                        scores_batch(hp, qb, eA, eB, k0, nk, half=1)
                        if 1 <= j <= 4:
                            mm7[j - 1]()  # a=0 MM chunks
                    for u in ev7[0:4]:
                        u()
                    for u in mm7[4:8]:
                        u()
                    for u in ev7[4:8]:
                        u()
                prev = (hp, qb, eA, eB)

    return nc


def _get_program(split_waits=True):
    key = ("nc", split_waits)
    if key not in _CACHE:
        _CACHE[key] = _build_program(split_waits)
    return _CACHE[key]


def _make_in_maps(hidden_states, attention_mask, Wq, bq, Wk, bk, Wv, bv):
    hidden = np.ascontiguousarray(np.asarray(hidden_states, dtype=np.float32))
    mask = np.asarray(attention_mask, dtype=np.float32)
    Wq = np.asarray(Wq, dtype=np.float32)
    Wk = np.asarray(Wk, dtype=np.float32)
    Wv = np.asarray(Wv, dtype=np.float32)
    bq = np.asarray(bq, dtype=np.float32)
    bk = np.asarray(bk, dtype=np.float32)
    bv = np.asarray(bv, dtype=np.float32)

    WqT = Wq.T  # [in, out]
    WkT = Wk.T
    WvT = Wv.T

    def pmajor(w):
        # [H, C] -> [128, FT*C] p-major flat, matching the SBUF tile
        c = w.shape[1]
        return np.ascontiguousarray(
            w.reshape(FT, 128, c).transpose(1, 0, 2).reshape(128, FT * c)
            .astype(np.float16))

    in_maps = []
    for c in range(NCORES):
        b, hg = divmod(c, HPC)
        cols = slice(hg * DS, (hg + 1) * DS)
        # xT: [p, nb, ft, j] flat [128, QB*FT*QBS]
        xT = hidden[b].T.astype(np.float16)  # [H, S]
        xT = np.ascontiguousarray(
            xT.reshape(FT, 128, QB, QBS).transpose(1, 2, 0, 3)
            .reshape(128, QB * FT * QBS))
        wqT = pmajor(WqT[:, cols])
        wkT = pmajor(WkT[:, cols])
        wv_base = WvT[:, cols]
        wvT = np.zeros((H, VW), np.float32)
        for hh in range(HPC):
            wvT[:, hh * (HD + 1):hh * (HD + 1) + HD] = \
                wv_base[:, hh * HD:(hh + 1) * HD]
        wvT = pmajor(wvT)
        bq_c = np.ascontiguousarray(bq[cols].reshape(2, 128).T)
        bk_c = np.ascontiguousarray(bk[cols].reshape(2, 128).T)
        bvb = np.ascontiguousarray(np.tile(bv[cols][None, :], (128, 1)))
        em = np.ascontiguousarray(
            np.exp(mask[b, 0, 0, :]).reshape(KT, 128).T.astype(np.float32))
        in_maps.append({
            "xT": xT, "wqT": wqT, "wkT": wkT, "wvT": wvT,
            "bq": bq_c, "bk": bk_c, "bvb": bvb, "em": em,
        })
    return in_maps


def _assemble(results):
    out = np.empty((B, S, H), np.float32)
    for c in range(NCORES):
        b, hg = divmod(c, HPC)
        # device layout [p, qt, j] -> [qt*128+p, j] = [S, DS]
        r = np.asarray(results[c]["out"]).reshape(128, 16, DS)
        out[b][:, hg * DS:(hg + 1) * DS] = \
            r.transpose(1, 0, 2).reshape(S, DS)
    return out


def _run(in_maps, trace=False):
    from concourse.bass_utils import run_bass_kernel_spmd
    nc = _get_program()
    return run_bass_kernel_spmd(
        nc, in_maps, core_ids=list(range(NCORES)), trace=trace)


def kernel(**inputs):
    in_maps = _make_in_maps(**inputs)
    res = _run(in_maps, trace=False)
    return _assemble(res.results)
